# revision 1
# baseline (speedup 1.0000x reference)
"""Trainium2 Bass kernel for the N^3 triplet descriptor (gnn_message_passing).

Strategy: the reference's O(N^3) angular sum factorizes exactly via the
Legendre addition theorem into O(N^2) per-pair vector moments:

  P0(cos) term: (sum_j w_j)^2
  P1(cos) term: |sum_j w_j u_j|^2                 (u = unit displacement)
  P2(cos) term: 1.5*|sum_j w_j u_j u_j^T|_F^2 - 0.5*(sum_j w_j)^2

with w_j = fc(r_ij) * r_ij^n.  Each device accumulates 45 pair moments per
central atom i (9 radial powers + 9 S1 components + 27 S2 components); the
tiny nonlinear combine runs on host after gathering.

Sharding: 8 cores = 2 i-blocks (96 rows on partitions) x 4 j-chunks (48
neighbors on the free axis). Cross-j-chunk partial sums are added on host.
"""

import numpy as np

import concourse.bass as bass
import concourse.bacc as bacc
import concourse.tile as tile
from concourse import mybir
from concourse.bass_utils import run_bass_kernel_spmd

F32 = mybir.dt.float32
N = 192
NI = 96          # i rows per core (partition dim)
NJ = 48          # j neighbors per core (free dim)
NIB = 2          # i blocks
NJC = 4          # j chunks
BOX_L = 20.0
RC = 5.0
HALF_PI = float(np.pi / 2)

_cached = {}


def build_nc():
    nc = bacc.Bacc(
        "TRN2",
        target_bir_lowering=False,
        debug=False,
        enable_asserts=True,
        num_devices=NIB * NJC,
    )
    # input: cols 0:144 = R[j_chunk].T replicated over partitions (d-major
    # blocks of 48), cols 144:147 = R[i_block] rows
    rji = nc.dram_tensor("rji", [NI, 160], F32, kind="ExternalInput").ap()
    out = nc.dram_tensor("out", [NI, 45], F32, kind="ExternalOutput").ap()

    with tile.TileContext(nc) as tc:
        with tc.tile_pool(name="p", bufs=1) as pool:
            rji_s = pool.tile([NI, 160], F32)
            nc.sync.dma_start(rji_s[:], rji)

            rj = rji_s[:, 0:144]                      # [96, 3*48] d-major
            ri = rji_s[:, 144:147]                    # [96, 3]
            rj3 = rj.rearrange("p (d j) -> p d j", d=3)
            ri3 = ri.unsqueeze(-1).broadcast_to((NI, 3, NJ))

            dxr = pool.tile([NI, 144], F32)           # raw Rj - Ri
            dxr3 = dxr.rearrange("p (d j) -> p d j", d=3)
            nc.vector.tensor_tensor(dxr3, rj3, ri3, op=mybir.AluOpType.subtract)

            # minimum image: dx -= 20*(dx > 10); dx += 20*(dx < -10)
            hi = pool.tile([NI, 144], F32)
            lo = pool.tile([NI, 144], F32)
            nc.vector.tensor_scalar(
                hi[:], dxr[:], BOX_L / 2, BOX_L,
                op0=mybir.AluOpType.is_gt, op1=mybir.AluOpType.mult,
            )
            nc.vector.tensor_scalar(
                lo[:], dxr[:], -BOX_L / 2, -BOX_L,
                op0=mybir.AluOpType.is_lt, op1=mybir.AluOpType.mult,
            )
            shift = pool.tile([NI, 144], F32)
            nc.vector.tensor_add(shift[:], hi[:], lo[:])
            dx = pool.tile([NI, 144], F32)
            nc.vector.tensor_sub(dx[:], dxr[:], shift[:])
            dx3 = dx.rearrange("p (d j) -> p d j", d=3)

            # r^2 then r
            sq = pool.tile([NI, 144], F32)
            nc.vector.tensor_mul(sq[:], dx[:], dx[:])
            r2 = pool.tile([NI, NJ], F32)
            nc.vector.reduce_sum(
                r2[:], sq.rearrange("p (d j) -> p j d", d=3),
                axis=mybir.AxisListType.X,
            )
            r = pool.tile([NI, NJ], F32)
            nc.scalar.sqrt(r[:], r2[:])

            # 1 / (r + 1e-8)
            rpe = pool.tile([NI, NJ], F32)
            nc.vector.tensor_scalar_add(rpe[:], r[:], 1e-8)
            rinv = pool.tile([NI, NJ], F32)
            nc.vector.reciprocal(rinv[:], rpe[:])
            rinv3 = rinv.unsqueeze(1).broadcast_to((NI, 3, NJ))

            # fc = sin^2((pi/2) * relu(1 - r/RC)) — exact 0 beyond cutoff
            t1 = pool.tile([NI, NJ], F32)
            nc.scalar.activation(
                t1[:], r[:], mybir.ActivationFunctionType.Relu,
                bias=1.0, scale=-1.0 / RC,
            )
            sn = pool.tile([NI, NJ], F32)
            nc.scalar.activation(
                sn[:], t1[:], mybir.ActivationFunctionType.Sin, scale=HALF_PI,
            )

            # fcp blocks k=0..8: fc * r^k
            fcp = pool.tile([NI, 9 * NJ], F32)
            nc.scalar.square(fcp[:, 0:NJ], sn[:])
            for k in range(1, 9):
                nc.vector.tensor_mul(
                    fcp[:, k * NJ:(k + 1) * NJ],
                    fcp[:, (k - 1) * NJ:k * NJ],
                    r[:],
                )

            sg = pool.tile([NI, 45], F32)
            # radial: q_r[k] = sum_j fc r^k
            nc.vector.reduce_sum(
                sg[:, 0:9], fcp.rearrange("p (k j) -> p k j", k=9),
                axis=mybir.AxisListType.X,
            )

            # v_n = fc r^n / (r+eps)  (n=0..2);  u_d = dx_d / (r+eps)
            v = pool.tile([NI, 144], F32)
            nc.vector.tensor_tensor(
                v.rearrange("p (n j) -> p n j", n=3),
                fcp[:, 0:144].rearrange("p (n j) -> p n j", n=3),
                rinv3, op=mybir.AluOpType.mult,
            )
            u = pool.tile([NI, 144], F32)
            nc.vector.tensor_tensor(
                u.rearrange("p (d j) -> p d j", d=3), dx3, rinv3,
                op=mybir.AluOpType.mult,
            )

            # t_{n,d} = v_n * dx_d = w_n u_d  -> [96, 9*48]
            t = pool.tile([NI, 9 * NJ], F32)
            nc.vector.tensor_tensor(
                t.rearrange("p (n d j) -> p n d j", n=3, d=3),
                v.rearrange("p (n j) -> p n j", n=3).unsqueeze(2).broadcast_to((NI, 3, 3, NJ)),
                dx.rearrange("p (d j) -> p d j", d=3).unsqueeze(1).broadcast_to((NI, 3, 3, NJ)),
                op=mybir.AluOpType.mult,
            )
            nc.vector.reduce_sum(
                sg[:, 9:18], t.rearrange("p (m j) -> p m j", m=9),
                axis=mybir.AxisListType.X,
            )

            # S2_{n,d,e} = sum_j t_{n,d} u_e -> 27 moments
            big = pool.tile([NI, 27 * NJ], F32)
            nc.vector.tensor_tensor(
                big.rearrange("p (m e j) -> p m e j", m=9, e=3),
                t.rearrange("p (m j) -> p m j", m=9).unsqueeze(2).broadcast_to((NI, 9, 3, NJ)),
                u.rearrange("p (e j) -> p e j", e=3).unsqueeze(1).broadcast_to((NI, 9, 3, NJ)),
                op=mybir.AluOpType.mult,
            )
            nc.vector.reduce_sum(
                sg[:, 18:45], big.rearrange("p (m j) -> p m j", m=27),
                axis=mybir.AxisListType.X,
            )

            nc.sync.dma_start(out, sg[:, 0:45])

    nc.compile()
    return nc


def host_prep(R):
    """Per-core input arrays: [96, 160] = [RjT replicated | Ri | pad]."""
    R = np.ascontiguousarray(R, np.float32)
    in_maps = []
    for core in range(NIB * NJC):
        ib, jc = divmod(core, NJC)
        rji = np.zeros((NI, 160), np.float32)
        rj = R[jc * NJ:(jc + 1) * NJ, :]              # [48, 3]
        rji[:, 0:144] = rj.T.reshape(1, 144)          # d-major, replicated
        rji[:, 144:147] = R[ib * NI:(ib + 1) * NI, :]
        in_maps.append({"rji": rji})
    return in_maps


def host_combine(partials):
    """partials: list of 8 [96,45] arrays (core order). Returns [192,18]."""
    sums = np.zeros((N, 45), np.float64)
    for core, p in enumerate(partials):
        ib = core // NJC
        sums[ib * NI:(ib + 1) * NI] += p.astype(np.float64)
    sums = sums.astype(np.float32)
    q_r = sums[:, 0:9].copy()
    q_r[:, 0] -= 1.0                                  # remove j==i self term
    s0 = q_r[:, 0:3]                                  # [N,3] n=0..2
    s1 = sums[:, 9:18].reshape(N, 3, 3)               # [N,n,d]
    s2 = sums[:, 18:45].reshape(N, 3, 3, 3)           # [N,n,d,e]
    ang = np.empty((N, 3, 3), np.float32)
    ang[:, :, 0] = s0 * s0
    ang[:, :, 1] = (s1 * s1).sum(-1)
    ang[:, :, 2] = 1.5 * (s2 * s2).sum((-1, -2)) - 0.5 * s0 * s0
    return np.concatenate([q_r, ang.reshape(N, 9)], axis=-1)


def _get_nc():
    if "nc" not in _cached:
        _cached["nc"] = build_nc()
    return _cached["nc"]


def kernel(R, box):
    R = np.asarray(R, np.float32)
    box = np.asarray(box, np.float32)
    assert R.shape == (N, 3)
    assert np.allclose(box, np.eye(3, dtype=np.float32) * BOX_L), (
        "kernel compiled for box = 20*I"
    )
    nc = _get_nc()
    in_maps = host_prep(R)
    res = run_bass_kernel_spmd(nc, in_maps, list(range(NIB * NJC)))
    partials = [res.results[c]["out"] for c in range(NIB * NJC)]
    return host_combine(partials)
